# revision 18
# baseline (speedup 1.0000x reference)
"""Trainium2 Bass kernel: GNN message passing  out = relu((adj @ x) @ W.T + b).

Reassociated as  out = relu(adj @ y + b)  with y = x @ W.T folded into host
prep (0.8% of the FLOPs), so the device runs a single big matmul.  That
matmul runs in fp8e4 DoubleRow mode (2 weights per PE cell, 256-deep
contraction per instruction, ~216 ns per [256x128]x[256x512] MM = fp8 peak).
fp8 error is held at ~1.8e-2 by two exact algebraic corrections folded into
the bias:
  * adj is mean-centered (B = adj - 0.5), halving its quantization noise;
    the rank-1 term 0.5 * colsum(y) is exact.
  * using colsum(y_true) rather than colsum(y_fp8) also cancels the
    coherent (mean) component of y's quantization error, halving it.

Sharding: 1D row partition of adj across 8 NeuronCores (1024 rows each);
each core computes outT = y.T @ adjT_c with y-tiles stationary and the
centered adj shard streaming, accumulating f32 in all 8 PSUM banks over
the 8192-deep contraction, then fuses bias+ReLU on PSUM eviction.

v6 structure (from NTFF profile iteration; the PE body is at the fp8
roofline, so head/tail/DMA-pacing are what's left):
  * y and adj are HOST-INTERLEAVED into one flat per-partition-contiguous
    stream in exact consumption order (per t-tile: y[2t:2t+2] then the two
    adj mc-halves, 3 KB/partition) and ride the single sync HWDGE queue as
    ~0.4-1.5 MB chunk DMAs.  One queue alone drains at full rate, arrival
    order is deterministic (the SDMA packet round-robin between queues is
    grossly unfair), and big transfers keep the cold-window rate high --
    the PE consumes 384 KB per 1.73 us t-tile (~222 GB/s) and the stream
    must outrun that from the start.
  * the DMA ring does not move bytes before ~8.2 us regardless of issue
    time; the first chunk is exactly tile 0 (384 KB) so the first real MM
    fires ~10.7 us.  18 warm-up matmuls on zeroed scratch keep the PE
    busy from ~8.0 us so the HAM clock gate is at 8/8 when real data
    lands (cold real MMs cost ~1 us regardless -- busy can't start before
    the preamble ends, so the 3.4 us HAM window can't close earlier).
  * tail: the last t-tile's 8 MMs stop banks in (n, mc) order; DVE evicts
    mc=0 (stops first), ACT mc=1, pairs staged per n.  Pairs n0-n2 store
    as 256 KB DMAs on sync; the critical last pair is split into two
    128 KB halves on sync+scalar, each issued the moment its half lands.
    No gpsimd DMAs anywhere: SWDGE drags the end-of-kernel drain ~2.5 us.
Output is stored bf16 to halve the post-matmul store drain.
"""

import numpy as np
import ml_dtypes

import concourse.mybir as mybir
from concourse import bacc
from concourse.tile import TileContext
from concourse.bass_utils import run_bass_kernel_spmd

P = 128
N_NODES = 8192
DIM = 512
NCORES = 8
M = N_NODES // NCORES          # 1024 output rows per core
KT = N_NODES // P              # 64 contraction tiles of 128
DT = KT // 2                   # 32 DoubleRow tiles (256-deep each)
NT = DIM // P                  # 4 tiles of the feature dim (output part.)
FREE = 512                     # moving free dim / PSUM bank width (f32)
MCH = M // FREE                # 2 moving chunks per t-tile
CB = 3 * 2 * FREE              # stream bytes/partition per t (y + 2 adj)
WARM = 28                      # HAM warm-up matmuls on scratch
FP8 = mybir.dt.float8e4
F32 = mybir.dt.float32
BF16 = mybir.dt.bfloat16
DR = mybir.MatmulPerfMode.DoubleRow

# chunk boundaries in t units: first chunk = exactly tile 0, then growing
# sizes (0.4 -> 1.5 MB) balancing arrival granularity vs transfer rate.
CHUNKS = [(0, 1), (1, 2), (2, 4), (4, 6), (6, 8), (8, 10), (10, 12),
          (12, 16), (16, 20), (20, 24), (24, 28), (28, 32)]

_NC = None


def _build_nc():
    nc = bacc.Bacc("TRN2", debug=False)
    # big[p, t, 0, i, d]    = y[(2t+i)*128+p, d]
    # big[p, t, 1+mc, i, m] = B.T[(2t+i)*128+p, mc*512+m]
    big_d = nc.dram_tensor("big", [P, DT * CB], FP8, kind="ExternalInput").ap()
    cb_d = nc.dram_tensor("cb", [P, NT], F32, kind="ExternalInput").ap()
    # out5[n, p, mc*512+r] = outT[n*128+p, mc*512+r]
    out_d = nc.dram_tensor(
        "out5", [NT, P, MCH * FREE], BF16, kind="ExternalOutput"
    ).ap()

    with TileContext(nc) as tc:
        with (
            tc.tile_pool(name="sb", bufs=1) as pool,
            tc.tile_pool(name="ps", bufs=8, space="PSUM") as pspool,
        ):
            str_sb = {}
            for ci, (t0, t1) in enumerate(CHUNKS):
                str_sb[ci] = pool.tile(
                    [P, t1 - t0, 3, 2, FREE], FP8, tag=f"str{ci}", name=f"str{ci}"
                )
            cb_sb = pool.tile([P, NT], F32, tag="cb")
            scr_sb = pool.tile([P, 2, 2 * P], FP8, tag="scr")
            o_sb = [
                pool.tile([P, MCH, FREE], BF16, tag=f"o{n}", name=f"o{n}")
                for n in range(NT)
            ]

            agg_ps = [
                [
                    pspool.tile([P, FREE], F32, tag="ps", name=f"ps_{mc}_{n}")
                    for n in range(NT)
                ]
                for mc in range(MCH)
            ]

            # The whole input stream rides the ONE sync HWDGE queue in
            # exact consumption order; bias rides behind it (needed only
            # by the epilogue).
            for ci, (t0, t1) in enumerate(CHUNKS):
                nc.sync.dma_start(str_sb[ci][:], big_d[:, t0 * CB : t1 * CB])
            nc.sync.dma_start(cb_sb[:], cb_d[:])

            # memset on gpsimd: it is otherwise idle and starts right at
            # main, so the PE's warm-up chain (and with it the 3.4 us HAM
            # busy window) starts ~0.5 us earlier than a DVE memset allows.
            nc.gpsimd.memset(scr_sb[:], 0)

            # Dummy matmuls on zeroed scratch keep the PE busy from the
            # end of its preamble so the HAM clock gate reaches 8/8 before
            # real data lands.  They open bank (0,0)'s accumulation group
            # and add exact zeros, so no group boundary (and no scheduler
            # wait) sits between them and the real stream.
            for w in range(WARM):
                nc.tensor.matmul(
                    agg_ps[0][0][:, 0:P],
                    scr_sb[:, :, :P],
                    scr_sb[:, :, P:],
                    start=(w == 0),
                    stop=False,
                    perf_mode=DR,
                )

            def t_chunk(t):
                for ci, (t0, t1) in enumerate(CHUNKS):
                    if t0 <= t < t1:
                        return ci, t0
                raise AssertionError

            for t in range(DT):
                ci, t0 = t_chunk(t)
                tile = str_sb[ci]
                j = t - t0
                for n in range(NT):
                    y_ap = tile[:, j, 0, :, n * P : (n + 1) * P]
                    for mc in range(MCH):
                        nc.tensor.matmul(
                            agg_ps[mc][n][:],
                            y_ap,
                            tile[:, j, 1 + mc],
                            start=(t == 0) and not (n == 0 and mc == 0),
                            stop=(t == DT - 1),
                            perf_mode=DR,
                        )

            # Epilogue: bias+ReLU on PSUM eviction.  Bank (mc, n) stops at
            # the (n, mc)-ordered last t-tile MMs; DVE takes mc=0 (stops
            # first), ACT mc=1.  Pairs n0-n2 store as one 256 KB HWDGE DMA
            # on sync (idle sequencer); the critical last pair splits into
            # two 128 KB halves on sync+scalar, each gated on only its own
            # eviction.  A store earlier in scalar's FIFO would stall
            # subsequent RELU dispatches; gpsimd (SWDGE) stores drag the
            # end-of-kernel drain by ~2.5 us.
            for n in range(NT):
                for mc in range(MCH):
                    dst = o_sb[n][:, mc]
                    if mc == 0:
                        nc.vector.tensor_scalar(
                            dst,
                            agg_ps[mc][n][:],
                            cb_sb[:, n : n + 1],
                            0.0,
                            mybir.AluOpType.add,
                            mybir.AluOpType.max,
                        )
                    else:
                        nc.scalar.activation(
                            dst,
                            agg_ps[mc][n][:],
                            mybir.ActivationFunctionType.Relu,
                            bias=cb_sb[:, n : n + 1],
                        )
                # Pairs n0-n2 ride gpsimd: they are issued ~5 us before the
                # kernel end, so their SWDGE completion is detected well
                # before the end barrier (no drain drag), and they keep the
                # sync HWDGE queue EMPTY so the critical last half-store's
                # bytes move immediately instead of FIFO-ing behind 768 KB.
                if n < NT - 1:
                    nc.gpsimd.dma_start(out_d[n], o_sb[n][:])
                else:
                    nc.sync.dma_start(out_d[n][:, 0:FREE], o_sb[n][:, 0])
                    nc.scalar.dma_start(out_d[n][:, FREE : 2 * FREE], o_sb[n][:, 1])
    nc.finalize()
    return nc


def _get_nc():
    global _NC
    if _NC is None:
        _NC = _build_nc()
    return _NC


def _prepare(inputs):
    e4 = ml_dtypes.float8_e4m3
    x = np.asarray(inputs["x"], dtype=np.float32)
    adj = np.asarray(inputs["adj"], dtype=np.float32)
    W = np.asarray(inputs["W"], dtype=np.float32)
    b = np.asarray(inputs["b"], dtype=np.float64)

    y = x @ W.T.astype(np.float32)
    y8 = y.astype(e4)
    # bias fold: nn bias + exact centering/rank-1 correction term
    c = (b + 0.5 * y.astype(np.float64).sum(axis=0)).astype(np.float32)
    cb_tiled = np.ascontiguousarray(c.reshape(NT, P).T)  # [128, 4]

    # y part of the stream: [p, t, i, d] = y8[(2t+i)*128+p, d]
    ypart = y8.reshape(DT, 2, P, DIM).transpose(2, 0, 1, 3)[:, :, None]

    B8T = (adj - np.float32(0.5)).astype(e4).T  # [K, rows] view

    in_maps = []
    for ci in range(NCORES):
        shard = np.ascontiguousarray(B8T[:, ci * M : (ci + 1) * M])
        # adj part: [p, t, mc, i, m] = B8T[(2t+i)*128+p, ci*M + mc*512+m]
        apart = shard.reshape(DT, 2, P, MCH, FREE).transpose(2, 0, 3, 1, 4)
        big = np.ascontiguousarray(
            np.concatenate([ypart, apart], axis=2)
        ).reshape(P, DT * CB)
        in_maps.append({"big": big, "cb": cb_tiled})
    return in_maps


def _run(in_maps, **kwargs):
    return run_bass_kernel_spmd(
        _get_nc(), in_maps, core_ids=list(range(NCORES)), **kwargs
    )


def _assemble(results):
    out = np.empty((N_NODES, DIM), dtype=np.float32)
    for ci in range(NCORES):
        o5 = results[ci]["out5"].astype(np.float32)  # [NT, P, MCH*FREE]
        # element [n, p, j] = out[ci*1024 + j, n*128 + p]
        blk = o5.transpose(2, 0, 1).reshape(M, DIM)
        out[ci * M : (ci + 1) * M, :] = blk
    return out


def kernel(**inputs):
    res = _run(_prepare(inputs))
    return _assemble(res.results)


# revision 20
# speedup vs baseline: 1.1852x; 1.1852x over previous
"""Trainium2 Bass kernel: GNN message passing  out = relu((adj @ x) @ W.T + b).

Reassociated as  out = relu(adj @ y + b)  with y = x @ W.T folded into host
prep (0.8% of the FLOPs), so the device runs a single big matmul.  That
matmul runs in fp8e4 DoubleRow mode (2 weights per PE cell, 256-deep
contraction per instruction, ~216 ns per [256x128]x[256x512] MM = fp8 peak).
fp8 error is held at ~1.8e-2 by two exact algebraic corrections folded into
the bias:
  * adj is mean-centered (B = adj - 0.5), halving its quantization noise;
    the rank-1 term 0.5 * colsum(y) is exact.
  * using colsum(y_true) rather than colsum(y_fp8) also cancels the
    coherent (mean) component of y's quantization error, halving it.

Sharding: 1D row partition of adj across 8 NeuronCores (1024 rows each);
each core computes outT = y.T @ adjT_c with y-tiles stationary and the
centered adj shard streaming, accumulating f32 in all 8 PSUM banks over
the 8192-deep contraction, then fuses bias+ReLU on PSUM eviction.

v6 structure (from NTFF profile iteration; the PE body is at the fp8
roofline, so head/tail/DMA-pacing are what's left):
  * y and adj are HOST-INTERLEAVED into one flat per-partition-contiguous
    stream in exact consumption order (per t-tile: y[2t:2t+2] then the two
    adj mc-halves, 3 KB/partition) and ride the single sync HWDGE queue as
    ~0.4-1.5 MB chunk DMAs.  One queue alone drains at full rate, arrival
    order is deterministic (the SDMA packet round-robin between queues is
    grossly unfair), and big transfers keep the cold-window rate high --
    the PE consumes 384 KB per 1.73 us t-tile (~222 GB/s) and the stream
    must outrun that from the start.
  * the DMA ring does not move bytes before ~8.2 us regardless of issue
    time; the first chunk is exactly tile 0 (384 KB) so the first real MM
    fires ~10.8-11.8 us.  28 warm-up matmuls on gpsimd-zeroed scratch
    keep the PE busy without a single idle gap from ~7.8 us until real
    data lands: the HAM clock gate needs one fully-busy free-running
    3.4 us window to reach 8/8, and ANY pre-flip idle gap restarts the
    accumulation (measured: a 300 ns gap cost six 427 ns cold MMs).
  * tail: the last t-tile's 8 MMs stop banks in (n, mc) order; DVE evicts
    mc=0 (stops first), ACT mc=1, pairs staged per n.  Pairs n0-n2 store
    as 256 KB DMAs on sync; the critical last pair is split into two
    128 KB halves on sync+scalar, each issued the moment its half lands.
    No gpsimd DMAs anywhere: SWDGE drags the end-of-kernel drain ~2.5 us.
Output is stored bf16 to halve the post-matmul store drain.
"""

import numpy as np
import ml_dtypes

import concourse.mybir as mybir
from concourse import bacc
from concourse.tile import TileContext
from concourse.bass_utils import run_bass_kernel_spmd

P = 128
N_NODES = 8192
DIM = 512
NCORES = 8
M = N_NODES // NCORES          # 1024 output rows per core
KT = N_NODES // P              # 64 contraction tiles of 128
DT = KT // 2                   # 32 DoubleRow tiles (256-deep each)
NT = DIM // P                  # 4 tiles of the feature dim (output part.)
FREE = 512                     # moving free dim / PSUM bank width (f32)
MCH = M // FREE                # 2 moving chunks per t-tile
CB = 3 * 2 * FREE              # stream bytes/partition per t (y + 2 adj)
WARM = 28                      # HAM warm-up matmuls on scratch
FP8 = mybir.dt.float8e4
F32 = mybir.dt.float32
BF16 = mybir.dt.bfloat16
DR = mybir.MatmulPerfMode.DoubleRow

# chunk boundaries in t units: first chunk = exactly tile 0, then growing
# sizes (0.4 -> 1.5 MB) balancing arrival granularity vs transfer rate.
CHUNKS = [(0, 1), (1, 2), (2, 4), (4, 6), (6, 8), (8, 10), (10, 12),
          (12, 16), (16, 20), (20, 24), (24, 28), (28, 32)]

_NC = None


def _build_nc():
    nc = bacc.Bacc("TRN2", debug=False)
    # big[p, t, 0, i, d]    = y[(2t+i)*128+p, d]
    # big[p, t, 1+mc, i, m] = B.T[(2t+i)*128+p, mc*512+m]
    big_d = nc.dram_tensor("big", [P, DT * CB], FP8, kind="ExternalInput").ap()
    cb_d = nc.dram_tensor("cb", [P, NT], F32, kind="ExternalInput").ap()
    # out5[n, p, mc*512+r] = outT[n*128+p, mc*512+r]
    out_d = nc.dram_tensor(
        "out5", [NT, P, MCH * FREE], BF16, kind="ExternalOutput"
    ).ap()

    with TileContext(nc) as tc:
        with (
            tc.tile_pool(name="sb", bufs=1) as pool,
            tc.tile_pool(name="ps", bufs=8, space="PSUM") as pspool,
        ):
            str_sb = {}
            for ci, (t0, t1) in enumerate(CHUNKS):
                str_sb[ci] = pool.tile(
                    [P, t1 - t0, 3, 2, FREE], FP8, tag=f"str{ci}", name=f"str{ci}"
                )
            cb_sb = pool.tile([P, NT], F32, tag="cb")
            scr_sb = pool.tile([P, 2, 2 * P], FP8, tag="scr")
            o_sb = [
                pool.tile([P, MCH, FREE], BF16, tag=f"o{n}", name=f"o{n}")
                for n in range(NT)
            ]

            agg_ps = [
                [
                    pspool.tile([P, FREE], F32, tag="ps", name=f"ps_{mc}_{n}")
                    for n in range(NT)
                ]
                for mc in range(MCH)
            ]

            # The whole input stream rides the ONE sync HWDGE queue in
            # exact consumption order; bias rides behind it (needed only
            # by the epilogue).
            for ci, (t0, t1) in enumerate(CHUNKS):
                nc.sync.dma_start(str_sb[ci][:], big_d[:, t0 * CB : t1 * CB])
            nc.sync.dma_start(cb_sb[:], cb_d[:])

            # memset on gpsimd: it is otherwise idle and starts right at
            # main, so the PE's warm-up chain (and with it the 3.4 us HAM
            # busy window) starts ~0.5 us earlier than a DVE memset allows.
            nc.gpsimd.memset(scr_sb[:], 0)

            # Dummy matmuls on zeroed scratch keep the PE busy from the
            # end of its preamble so the HAM clock gate reaches 8/8 before
            # real data lands.  They open bank (0,0)'s accumulation group
            # and add exact zeros, so no group boundary (and no scheduler
            # wait) sits between them and the real stream.
            for w in range(WARM):
                nc.tensor.matmul(
                    agg_ps[0][0][:, 0:P],
                    scr_sb[:, :, :P],
                    scr_sb[:, :, P:],
                    start=(w == 0),
                    stop=False,
                    perf_mode=DR,
                )

            def t_chunk(t):
                for ci, (t0, t1) in enumerate(CHUNKS):
                    if t0 <= t < t1:
                        return ci, t0
                raise AssertionError

            for t in range(DT):
                ci, t0 = t_chunk(t)
                tile = str_sb[ci]
                j = t - t0
                for n in range(NT):
                    y_ap = tile[:, j, 0, :, n * P : (n + 1) * P]
                    for mc in range(MCH):
                        nc.tensor.matmul(
                            agg_ps[mc][n][:],
                            y_ap,
                            tile[:, j, 1 + mc],
                            start=(t == 0) and not (n == 0 and mc == 0),
                            stop=(t == DT - 1),
                            perf_mode=DR,
                        )

            # Epilogue: bias+ReLU on PSUM eviction.  Bank (mc, n) stops at
            # the (n, mc)-ordered last t-tile MMs; DVE takes mc=0 (stops
            # first), ACT mc=1.  Pairs n0-n2 store as one 256 KB HWDGE DMA
            # on sync (idle sequencer); the critical last pair splits into
            # two 128 KB halves on sync+scalar, each gated on only its own
            # eviction.  A store earlier in scalar's FIFO would stall
            # subsequent RELU dispatches; gpsimd (SWDGE) stores drag the
            # end-of-kernel drain by ~2.5 us.
            for n in range(NT):
                for mc in range(MCH):
                    dst = o_sb[n][:, mc]
                    if mc == 0:
                        nc.vector.tensor_scalar(
                            dst,
                            agg_ps[mc][n][:],
                            cb_sb[:, n : n + 1],
                            0.0,
                            mybir.AluOpType.add,
                            mybir.AluOpType.max,
                        )
                    else:
                        nc.scalar.activation(
                            dst,
                            agg_ps[mc][n][:],
                            mybir.ActivationFunctionType.Relu,
                            bias=cb_sb[:, n : n + 1],
                        )
                if n < NT - 1:
                    nc.sync.dma_start(out_d[n], o_sb[n][:])
                else:
                    nc.sync.dma_start(out_d[n][:, 0:FREE], o_sb[n][:, 0])
                    nc.scalar.dma_start(out_d[n][:, FREE : 2 * FREE], o_sb[n][:, 1])
    nc.finalize()
    return nc


def _get_nc():
    global _NC
    if _NC is None:
        _NC = _build_nc()
    return _NC


def _prepare(inputs):
    e4 = ml_dtypes.float8_e4m3
    x = np.asarray(inputs["x"], dtype=np.float32)
    adj = np.asarray(inputs["adj"], dtype=np.float32)
    W = np.asarray(inputs["W"], dtype=np.float32)
    b = np.asarray(inputs["b"], dtype=np.float64)

    y = x @ W.T.astype(np.float32)
    y8 = y.astype(e4)
    # bias fold: nn bias + exact centering/rank-1 correction term
    c = (b + 0.5 * y.astype(np.float64).sum(axis=0)).astype(np.float32)
    cb_tiled = np.ascontiguousarray(c.reshape(NT, P).T)  # [128, 4]

    # y part of the stream: [p, t, i, d] = y8[(2t+i)*128+p, d]
    ypart = y8.reshape(DT, 2, P, DIM).transpose(2, 0, 1, 3)[:, :, None]

    B8T = (adj - np.float32(0.5)).astype(e4).T  # [K, rows] view

    in_maps = []
    for ci in range(NCORES):
        shard = np.ascontiguousarray(B8T[:, ci * M : (ci + 1) * M])
        # adj part: [p, t, mc, i, m] = B8T[(2t+i)*128+p, ci*M + mc*512+m]
        apart = shard.reshape(DT, 2, P, MCH, FREE).transpose(2, 0, 3, 1, 4)
        big = np.ascontiguousarray(
            np.concatenate([ypart, apart], axis=2)
        ).reshape(P, DT * CB)
        in_maps.append({"big": big, "cb": cb_tiled})
    return in_maps


def _run(in_maps, **kwargs):
    return run_bass_kernel_spmd(
        _get_nc(), in_maps, core_ids=list(range(NCORES)), **kwargs
    )


def _assemble(results):
    out = np.empty((N_NODES, DIM), dtype=np.float32)
    for ci in range(NCORES):
        o5 = results[ci]["out5"].astype(np.float32)  # [NT, P, MCH*FREE]
        # element [n, p, j] = out[ci*1024 + j, n*128 + p]
        blk = o5.transpose(2, 0, 1).reshape(M, DIM)
        out[ci * M : (ci + 1) * M, :] = blk
    return out


def kernel(**inputs):
    res = _run(_prepare(inputs))
    return _assemble(res.results)


# revision 24
# speedup vs baseline: 1.2183x; 1.0280x over previous
"""Trainium2 Bass kernel: GNN message passing  out = relu((adj @ x) @ W.T + b).

Reassociated as  out = relu(adj @ y + b)  with y = x @ W.T folded into host
prep (0.8% of the FLOPs), so the device runs a single big matmul.  That
matmul runs in fp8e4 DoubleRow mode (2 weights per PE cell, 256-deep
contraction per instruction, ~216 ns per [256x128]x[256x512] MM = fp8 peak).
fp8 error is held at ~1.8e-2 by two exact algebraic corrections folded into
the bias:
  * adj is mean-centered (B = adj - 0.5), halving its quantization noise;
    the rank-1 term 0.5 * colsum(y) is exact.
  * using colsum(y_true) rather than colsum(y_fp8) also cancels the
    coherent (mean) component of y's quantization error, halving it.

Sharding: 1D row partition of adj across 8 NeuronCores (1024 rows each);
each core computes outT = y.T @ adjT_c with y-tiles stationary and the
centered adj shard streaming, accumulating f32 in all 8 PSUM banks over
the 8192-deep contraction, then fuses bias+ReLU on PSUM eviction.

v6 structure (from NTFF profile iteration; the PE body is at the fp8
roofline, so head/tail/DMA-pacing are what's left):
  * y and adj are HOST-INTERLEAVED into one flat per-partition-contiguous
    stream in exact consumption order (per t-tile: y[2t:2t+2] then the two
    adj mc-halves, 3 KB/partition) and ride the single sync HWDGE queue as
    ~0.4-1.5 MB chunk DMAs.  One queue alone drains at full rate, arrival
    order is deterministic (the SDMA packet round-robin between queues is
    grossly unfair), and big transfers keep the cold-window rate high --
    the PE consumes 384 KB per 1.73 us t-tile (~222 GB/s) and the stream
    must outrun that from the start.
  * the DMA ring does not move bytes before ~8.2 us regardless of issue
    time; the first chunk is exactly tile 0 (384 KB) so the first real MM
    fires ~10.8-11.8 us.  28 warm-up matmuls on gpsimd-zeroed scratch
    keep the PE busy without a single idle gap from ~7.8 us until real
    data lands: the HAM clock gate needs one fully-busy free-running
    3.4 us window to reach 8/8, and ANY pre-flip idle gap restarts the
    accumulation (measured: a 300 ns gap cost six 427 ns cold MMs).
  * tail: the last t-tile's 8 MMs stop banks in (n, mc) order; DVE evicts
    mc=0 (stops first), ACT mc=1, pairs staged per n.  Pairs n0-n2 store
    as 256 KB DMAs on sync; the critical last pair is split into two
    128 KB halves on sync+scalar, each issued the moment its half lands.
    No gpsimd DMAs anywhere: SWDGE drags the end-of-kernel drain ~2.5 us.
Output is stored bf16 to halve the post-matmul store drain.
"""

import numpy as np
import ml_dtypes

import concourse.mybir as mybir
from concourse import bacc
from concourse.tile import TileContext
from concourse.bass_utils import run_bass_kernel_spmd

P = 128
N_NODES = 8192
DIM = 512
NCORES = 8
M = N_NODES // NCORES          # 1024 output rows per core
KT = N_NODES // P              # 64 contraction tiles of 128
DT = KT // 2                   # 32 DoubleRow tiles (256-deep each)
NT = DIM // P                  # 4 tiles of the feature dim (output part.)
FREE = 512                     # moving free dim / PSUM bank width (f32)
MCH = M // FREE                # 2 moving chunks per t-tile
CB = 3 * 2 * FREE              # stream bytes/partition per t (y + 2 adj)
WARM = 28                      # HAM warm-up matmuls on scratch
FP8 = mybir.dt.float8e4
F32 = mybir.dt.float32
BF16 = mybir.dt.bfloat16
DR = mybir.MatmulPerfMode.DoubleRow

# chunk boundaries in t units (tile 0 is loaded separately as two tiles:
# y+adj(mc0) gating the first 4 MMs, adj(mc1) behind it), then growing
# sizes (0.4 -> 1.5 MB) balancing arrival granularity vs transfer rate.
CHUNKS = [(1, 2), (2, 4), (4, 6), (6, 8), (8, 10), (10, 12),
          (12, 16), (16, 20), (20, 24), (24, 28), (28, 32)]
# The last TAIL_T tiles' MMs are grouped per-bank so the 8 PSUM banks
# stop staggered over 24 MMs (5.2 us) instead of the last 8 (1.7 us):
# each ~0.6 us eviction then pipelines with zero queueing and only the
# final bank's eviction sits after the last MM (same MMs, same per-bank
# t-ascending accumulation order, so the result is bit-identical).
TAIL_T = 3

_NC = None


def _build_nc():
    nc = bacc.Bacc("TRN2", debug=False)
    # big[p, t, 0, i, d]    = y[(2t+i)*128+p, d]
    # big[p, t, 1+mc, i, m] = B.T[(2t+i)*128+p, mc*512+m]
    big_d = nc.dram_tensor("big", [P, DT * CB], FP8, kind="ExternalInput").ap()
    cb_d = nc.dram_tensor("cb", [P, NT], F32, kind="ExternalInput").ap()
    # out5[n, p, mc*512+r] = outT[n*128+p, mc*512+r]
    out_d = nc.dram_tensor(
        "out5", [NT, P, MCH * FREE], BF16, kind="ExternalOutput"
    ).ap()

    with TileContext(nc) as tc:
        with (
            tc.tile_pool(name="sb", bufs=1) as pool,
            tc.tile_pool(name="ps", bufs=8, space="PSUM") as pspool,
        ):
            # tile 0 split into two tiles so the first 4 MMs are gated on
            # only y(t0)+adj(t0,mc0) = 256 KB.
            str0a = pool.tile([P, 2, 2, FREE], FP8, tag="str0a")
            str0b = pool.tile([P, 2, FREE], FP8, tag="str0b")
            str_sb = {}
            for ci, (t0, t1) in enumerate(CHUNKS):
                str_sb[ci] = pool.tile(
                    [P, t1 - t0, 3, 2, FREE], FP8, tag=f"str{ci}", name=f"str{ci}"
                )
            cb_sb = pool.tile([P, NT], F32, tag="cb")
            scr_sb = pool.tile([P, 2, 2 * P], FP8, tag="scr")
            o_sb = [
                pool.tile([P, MCH, FREE], BF16, tag=f"o{n}", name=f"o{n}")
                for n in range(NT)
            ]

            agg_ps = [
                [
                    pspool.tile([P, FREE], F32, tag="ps", name=f"ps_{mc}_{n}")
                    for n in range(NT)
                ]
                for mc in range(MCH)
            ]

            # The whole input stream rides the ONE sync HWDGE queue in
            # exact consumption order; bias rides behind it (needed only
            # by the epilogue).
            nc.sync.dma_start(str0a[:], big_d[:, 0 : 2 * 2 * FREE])
            nc.sync.dma_start(str0b[:], big_d[:, 2 * 2 * FREE : CB])
            for ci, (t0, t1) in enumerate(CHUNKS):
                nc.sync.dma_start(str_sb[ci][:], big_d[:, t0 * CB : t1 * CB])
            nc.sync.dma_start(cb_sb[:], cb_d[:])

            # memset on gpsimd: it is otherwise idle and starts right at
            # main, so the PE's warm-up chain (and with it the 3.4 us HAM
            # busy window) starts ~0.5 us earlier than a DVE memset allows.
            nc.gpsimd.memset(scr_sb[:], 0)

            # Dummy matmuls on zeroed scratch keep the PE busy from the
            # end of its preamble so the HAM clock gate reaches 8/8 before
            # real data lands.  They open bank (0,0)'s accumulation group
            # and add exact zeros, so no group boundary (and no scheduler
            # wait) sits between them and the real stream.
            for w in range(WARM):
                nc.tensor.matmul(
                    agg_ps[0][0][:, 0:P],
                    scr_sb[:, :, :P],
                    scr_sb[:, :, P:],
                    start=(w == 0),
                    stop=False,
                    perf_mode=DR,
                )

            def t_chunk(t):
                for ci, (t0, t1) in enumerate(CHUNKS):
                    if t0 <= t < t1:
                        return ci, t0
                raise AssertionError

            def aps(t, n, mc):
                if t == 0:
                    y_ap = str0a[:, 0, :, n * P : (n + 1) * P]
                    a_ap = str0a[:, 1] if mc == 0 else str0b[:]
                else:
                    ci, t0 = t_chunk(t)
                    j = t - t0
                    y_ap = str_sb[ci][:, j, 0, :, n * P : (n + 1) * P]
                    a_ap = str_sb[ci][:, j, 1 + mc]
                return y_ap, a_ap

            for t in range(DT - TAIL_T):
                for n in range(NT):
                    for mc in range(MCH):
                        y_ap, a_ap = aps(t, n, mc)
                        nc.tensor.matmul(
                            agg_ps[mc][n][:],
                            y_ap,
                            a_ap,
                            start=(t == 0) and not (n == 0 and mc == 0),
                            stop=False,
                            perf_mode=DR,
                        )
            # staggered tail: per-bank grouping of the last TAIL_T tiles
            for n in range(NT):
                for mc in range(MCH):
                    for t in range(DT - TAIL_T, DT):
                        y_ap, a_ap = aps(t, n, mc)
                        nc.tensor.matmul(
                            agg_ps[mc][n][:],
                            y_ap,
                            a_ap,
                            start=False,
                            stop=(t == DT - 1),
                            perf_mode=DR,
                        )

            # Epilogue: bias+ReLU on PSUM eviction.  Bank (mc, n) stops at
            # the (n, mc)-ordered last t-tile MMs; DVE takes mc=0 (stops
            # first), ACT mc=1.  Pairs n0-n2 store as one 256 KB HWDGE DMA
            # on sync (idle sequencer); the critical last pair splits into
            # two 128 KB halves on sync+scalar, each gated on only its own
            # eviction.  A store earlier in scalar's FIFO would stall
            # subsequent RELU dispatches; gpsimd (SWDGE) stores drag the
            # end-of-kernel drain by ~2.5 us.
            for n in range(NT):
                for mc in range(MCH):
                    dst = o_sb[n][:, mc]
                    if mc == 0:
                        nc.vector.tensor_scalar(
                            dst,
                            agg_ps[mc][n][:],
                            cb_sb[:, n : n + 1],
                            0.0,
                            mybir.AluOpType.add,
                            mybir.AluOpType.max,
                        )
                    else:
                        nc.scalar.activation(
                            dst,
                            agg_ps[mc][n][:],
                            mybir.ActivationFunctionType.Relu,
                            bias=cb_sb[:, n : n + 1],
                        )
                if n < NT - 1:
                    nc.sync.dma_start(out_d[n], o_sb[n][:])
                else:
                    nc.sync.dma_start(out_d[n][:, 0:FREE], o_sb[n][:, 0])
                    nc.scalar.dma_start(out_d[n][:, FREE : 2 * FREE], o_sb[n][:, 1])
    nc.finalize()
    return nc


def _get_nc():
    global _NC
    if _NC is None:
        _NC = _build_nc()
    return _NC


def _prepare(inputs):
    e4 = ml_dtypes.float8_e4m3
    x = np.asarray(inputs["x"], dtype=np.float32)
    adj = np.asarray(inputs["adj"], dtype=np.float32)
    W = np.asarray(inputs["W"], dtype=np.float32)
    b = np.asarray(inputs["b"], dtype=np.float64)

    y = x @ W.T.astype(np.float32)
    y8 = y.astype(e4)
    # bias fold: nn bias + exact centering/rank-1 correction term
    c = (b + 0.5 * y.astype(np.float64).sum(axis=0)).astype(np.float32)
    cb_tiled = np.ascontiguousarray(c.reshape(NT, P).T)  # [128, 4]

    # y part of the stream: [p, t, i, d] = y8[(2t+i)*128+p, d]
    ypart = y8.reshape(DT, 2, P, DIM).transpose(2, 0, 1, 3)[:, :, None]

    B8T = (adj - np.float32(0.5)).astype(e4).T  # [K, rows] view

    in_maps = []
    for ci in range(NCORES):
        shard = np.ascontiguousarray(B8T[:, ci * M : (ci + 1) * M])
        # adj part: [p, t, mc, i, m] = B8T[(2t+i)*128+p, ci*M + mc*512+m]
        apart = shard.reshape(DT, 2, P, MCH, FREE).transpose(2, 0, 3, 1, 4)
        big = np.ascontiguousarray(
            np.concatenate([ypart, apart], axis=2)
        ).reshape(P, DT * CB)
        in_maps.append({"big": big, "cb": cb_tiled})
    return in_maps


def _run(in_maps, **kwargs):
    return run_bass_kernel_spmd(
        _get_nc(), in_maps, core_ids=list(range(NCORES)), **kwargs
    )


def _assemble(results):
    out = np.empty((N_NODES, DIM), dtype=np.float32)
    for ci in range(NCORES):
        o5 = results[ci]["out5"].astype(np.float32)  # [NT, P, MCH*FREE]
        # element [n, p, j] = out[ci*1024 + j, n*128 + p]
        blk = o5.transpose(2, 0, 1).reshape(M, DIM)
        out[ci * M : (ci + 1) * M, :] = blk
    return out


def kernel(**inputs):
    res = _run(_prepare(inputs))
    return _assemble(res.results)
